# revision 3
# baseline (speedup 1.0000x reference)
"""GuidedFilter Trainium2 kernel: batch-parallel over 8 NeuronCores.

Per core: img [1,512,512] f32, feat [16,512,512] f32 -> out [16,512,512] f32.
Each 2-D reflect box blur (radius 5) is two TensorE passes against a banded
unnormalized box matrix B (entries {0,1,2}, exact in bf16):
  pass A' (data-as-weights): T1 = X^T B^T   (contracts partition dim, flips orientation)
  pass C  (const weights):   out = B T1     (contracts partition dim again)
=> out = B X^T... = (B X B^T)^T = raw 2-D blur, transposed. The 1/121
normalization is folded into later elementwise ops. Orientations alternate so
no explicit transposes are needed anywhere.
"""
import sys

sys.path.insert(0, "/opt/trn_rl_repo")

import numpy as np
import ml_dtypes

RADIUS = 5
EPS = 1e-08
H = W = 512
D = 16
NCORES = 8
U = 1.0 / 121.0  # box normalization (11x11)

_BAND = [ [0, 1], [0, 1, 2], [1, 2, 3], [2, 3] ]  # band(j): i-blocks touched
_GJ_OFF = [0, 256, 640, 1024]                      # col offset of GJ[j] in packed G
_GJ_LEN = [256, 384, 384, 256]


def _box_matrix():
    B = np.zeros((512, 512), np.float32)
    for i in range(512):
        for d in range(-RADIUS, RADIUS + 1):
            j = i + d
            if j < 0:
                j = -j
            elif j > 511:
                j = 1022 - j
            B[i, j] += 1.0
    return B


def _g_packed():
    B = _box_matrix()
    cols = []
    for j in range(4):
        for i in _BAND[j]:
            cols.append(B[128 * i:128 * i + 128, 128 * j:128 * j + 128].T)
    return np.ascontiguousarray(np.concatenate(cols, axis=1)).astype(ml_dtypes.bfloat16)


def _build_bass():
    import concourse.bass as bass
    import concourse.bacc as bacc
    import concourse.tile as tile
    from concourse import mybir

    f32 = mybir.dt.float32
    bf16 = mybir.dt.bfloat16
    Alu = mybir.AluOpType
    Act = mybir.ActivationFunctionType

    nc = bacc.Bacc("TRN2", target_bir_lowering=False, debug=False,
                   num_devices=NCORES)

    feat_d = nc.dram_tensor("feat", [D, H, W], f32, kind="ExternalInput").ap()
    img_d = nc.dram_tensor("img", [1, H, W], f32, kind="ExternalInput").ap()
    g_d = nc.dram_tensor("gmat", [128, 1280], bf16, kind="ExternalInput").ap()
    out_d = nc.dram_tensor("out", [D, H, W], f32, kind="ExternalOutput").ap()

    def ld(dst, src2d):
        # HBM [512,512] f32 -> SBUF [128, 4*512] (j-chunk major), cast to bf16
        nc.gpsimd.dma_start(
            out=dst.rearrange("p (j w) -> p j w", j=4),
            in_=src2d.rearrange("(j p) w -> p j w", p=128))

    with tile.TileContext(nc) as tc:
        with (
            tc.tile_pool(name="consts", bufs=1) as consts,
            tc.tile_pool(name="shared", bufs=1) as shared,
            tc.tile_pool(name="chan", bufs=2) as chan,
            tc.tile_pool(name="psum", bufs=1, space="PSUM") as psum,
        ):
            G = consts.tile([128, 1280], bf16)
            nc.gpsimd.dma_start(out=G[:], in_=g_d)
            I = consts.tile([128, 2048], bf16)
            ld(I, img_d[0])

            def ap_blur(X):
                """pass A': psum tiles T1 (list of 4 [128,512] f32) = X^T B^T."""
                ps = [psum.tile([128, 512], f32, name=f"psA{wb}", tag=f"psA{wb}") for wb in range(4)]
                for j in range(4):
                    i0 = _BAND[j][0]
                    ilen = len(_BAND[j])
                    rhs = G[:, _GJ_OFF[j]:_GJ_OFF[j] + _GJ_LEN[j]]
                    for wb in range(4):
                        lhsT = X[:, 512 * j + 128 * wb: 512 * j + 128 * (wb + 1)]
                        nc.tensor.matmul(
                            ps[wb][:, 128 * i0: 128 * (i0 + ilen)],
                            lhsT, rhs, start=(j == 0), stop=(j == 3),
                            skip_group_check=True)
                return ps

            def handoff(ps, engines="AADD"):
                """psum A' tiles -> one [128,2048] bf16 sbuf tile."""
                t = chan.tile([128, 2048], bf16, tag="t1")
                for wb in range(4):
                    dst = t[:, 512 * wb: 512 * (wb + 1)]
                    if engines[wb] == "A":
                        nc.scalar.copy(dst, ps[wb][:])
                    else:
                        nc.vector.tensor_copy(dst, ps[wb][:])
                return t

            def c_blur(T1, tag):
                """pass C: psum tiles out[i] [128,512] f32 = (B @ T1) chunks."""
                ps = [psum.tile([128, 512], f32, name=f"psC{i}", tag=f"psC{i}") for i in range(4)]
                for i in range(4):
                    band = _BAND[i]  # symmetric: band(i) as j-range equals _BAND[i]
                    for pos, j in enumerate(band):
                        off = _GJ_OFF[j] + 128 * (i - _BAND[j][0])
                        lhsT = G[:, off:off + 128]
                        rhs = T1[:, 512 * j: 512 * (j + 1)]
                        nc.tensor.matmul(
                            ps[i][:], lhsT, rhs,
                            start=(pos == 0), stop=(pos == len(band) - 1))
                return ps

            def blur2(X):
                return c_blur(handoff(ap_blur(X)), tag="c")

            # ---- shared (img) stage; all blur outputs are [W,H]-oriented ----
            I2 = shared.tile([128, 2048], bf16)
            nc.vector.tensor_mul(I2[:], I[:], I[:])
            psI = blur2(I)   # mI_raw^T
            mIs = shared.tile([128, 2048], bf16)   # true-scale mean_I^T
            for wb in range(4):
                nc.scalar.activation(mIs[:, 512 * wb:512 * (wb + 1)], psI[wb][:],
                                     Act.Copy, 0.0, U)
            psI2 = blur2(I2)  # corrI_raw^T
            m2 = shared.tile([128, 2048], f32)
            nc.vector.tensor_mul(m2[:], mIs[:], mIs[:])
            varps = shared.tile([128, 2048], f32)
            for wb in range(4):
                sl = slice(512 * wb, 512 * (wb + 1))
                # var = U*corrI_raw - mI^2   (merged psum evac)
                nc.vector.scalar_tensor_tensor(
                    varps[:, sl], psI2[wb][:], U, m2[:, sl],
                    op0=Alu.mult, op1=Alu.subtract)
            nc.vector.tensor_scalar_add(varps[:], varps[:], EPS)
            R = shared.tile([128, 2048], f32)
            nc.vector.reciprocal_approx_fast(R[:], varps[:])
            RS = shared.tile([128, 2048], bf16)
            nc.vector.tensor_scalar_mul(RS[:], R[:], U)     # U * R
            mIR = shared.tile([128, 2048], bf16)
            nc.vector.tensor_mul(mIR[:], mIs[:], R[:])      # mI * R

            # ---- per feature channel ----
            for d in range(D):
                Xd = chan.tile([128, 2048], bf16, tag="xd")
                ld(Xd, feat_d[d])
                Pd = chan.tile([128, 2048], bf16, tag="pd")
                nc.vector.tensor_mul(Pd[:], Xd[:], I[:])

                ps_mp = blur2(Xd)       # mp_raw^T
                mp = chan.tile([128, 2048], bf16, tag="mp")
                for wb in range(4):     # true-scale mp (ACT, scale=U)
                    nc.scalar.activation(mp[:, 512 * wb:512 * (wb + 1)],
                                         ps_mp[wb][:], Act.Copy, 0.0, U)
                ps_cip = blur2(Pd)      # corrIp_raw^T
                t2 = chan.tile([128, 2048], bf16, tag="t2")
                for wb in range(4):     # t2 = corrIp * R  (merged evac)
                    sl = slice(512 * wb, 512 * (wb + 1))
                    nc.vector.tensor_mul(t2[:, sl], ps_cip[wb][:], RS[:, sl])
                t1 = chan.tile([128, 2048], bf16, tag="t1m")
                nc.vector.tensor_mul(t1[:], mp[:], mIR[:])
                a = chan.tile([128, 2048], bf16, tag="a")
                nc.vector.tensor_sub(a[:], t2[:], t1[:])
                u2 = chan.tile([128, 2048], bf16, tag="u2")
                nc.vector.tensor_mul(u2[:], a[:], mIs[:])
                b = chan.tile([128, 2048], bf16, tag="b")
                nc.vector.tensor_sub(b[:], mp[:], u2[:])

                ps_ma = blur2(a)        # ma_raw, [H,W] again
                v = chan.tile([128, 2048], f32, tag="v")
                for wb in range(4):     # v = (U^2 * ma_raw) * I   (a true-scale => /121 once... )
                    sl = slice(512 * wb, 512 * (wb + 1))
                    nc.vector.scalar_tensor_tensor(
                        v[:, sl], ps_ma[wb][:], U, I[:, sl],
                        op0=Alu.mult, op1=Alu.mult)
                ps_mb = blur2(b)        # mb_raw
                o = chan.tile([128, 2048], bf16, tag="o")
                for wb in range(4):     # o = U*mb_raw + v
                    sl = slice(512 * wb, 512 * (wb + 1))
                    nc.vector.scalar_tensor_tensor(
                        o[:, sl], ps_mb[wb][:], U, v[:, sl],
                        op0=Alu.mult, op1=Alu.add)
                nc.gpsimd.dma_start(
                    out=out_d[d].rearrange("(j p) w -> p j w", p=128),
                    in_=o.rearrange("p (j w) -> p j w", j=4))

    nc.compile()
    return nc


_NC_CACHE = None


def kernel(feat: np.ndarray, img: np.ndarray) -> np.ndarray:
    global _NC_CACHE
    from concourse.bass_utils import run_bass_kernel_spmd

    if _NC_CACHE is None:
        _NC_CACHE = _build_bass()
    nc = _NC_CACHE
    g = _g_packed()
    feat = np.asarray(feat, np.float32)
    img = np.asarray(img, np.float32)
    in_maps = [
        {"feat": feat[c], "img": img[c], "gmat": g} for c in range(NCORES)
    ]
    res = run_bass_kernel_spmd(nc, in_maps, list(range(NCORES)))
    return np.stack([res.results[c]["out"] for c in range(NCORES)], axis=0)


# revision 8
# speedup vs baseline: 15749.6840x; 15749.6840x over previous
"""GuidedFilter Trainium2 kernel: batch-parallel over 8 NeuronCores.

Per core: img [1,512,512] f32, feat [16,512,512] f32 -> out [16,512,512] f32.
Each 2-D reflect box blur (radius 5) is two TensorE passes against a banded
unnormalized box matrix B (entries {0,1,2}, exact in bf16):
  pass A' (data-as-weights): T1 = X^T B^T   (contracts partition dim, flips orientation)
  pass C  (const weights):   out = B T1     (contracts partition dim again)
=> out = B X^T... = (B X B^T)^T = raw 2-D blur, transposed. The 1/121
normalization is folded into later elementwise ops. Orientations alternate so
no explicit transposes are needed anywhere.
"""
import sys

sys.path.insert(0, "/opt/trn_rl_repo")

import numpy as np
import ml_dtypes

RADIUS = 5
EPS = 1e-08
H = W = 512
D = 16
NCORES = 8
U = 1.0 / 121.0  # box normalization (11x11)

_BAND = [ [0, 1], [0, 1, 2], [1, 2, 3], [2, 3] ]  # band(j): i-blocks touched
_GJ_OFF = [0, 256, 640, 1024]                      # col offset of GJ[j] in packed G
_GJ_LEN = [256, 384, 384, 256]


def _box_matrix():
    B = np.zeros((512, 512), np.float32)
    for i in range(512):
        for d in range(-RADIUS, RADIUS + 1):
            j = i + d
            if j < 0:
                j = -j
            elif j > 511:
                j = 1022 - j
            B[i, j] += 1.0
    return B


def _g_packed():
    B = _box_matrix()
    cols = []
    for j in range(4):
        for i in _BAND[j]:
            cols.append(B[128 * i:128 * i + 128, 128 * j:128 * j + 128].T)
    return np.ascontiguousarray(np.concatenate(cols, axis=1)).astype(ml_dtypes.bfloat16)


def _build_bass():
    import concourse.bass as bass
    import concourse.bacc as bacc
    import concourse.tile as tile
    from concourse import mybir

    f32 = mybir.dt.float32
    bf16 = mybir.dt.bfloat16
    Alu = mybir.AluOpType
    Act = mybir.ActivationFunctionType

    nc = bacc.Bacc("TRN2", target_bir_lowering=False, debug=False,
                   num_devices=NCORES)

    feat_d = nc.dram_tensor("feat", [D, H, W], f32, kind="ExternalInput").ap()
    img_d = nc.dram_tensor("img", [1, H, W], f32, kind="ExternalInput").ap()
    g_d = nc.dram_tensor("gmat", [128, 1280], bf16, kind="ExternalInput").ap()
    out_d = nc.dram_tensor("out", [D, H, W], f32, kind="ExternalOutput").ap()

    def ld(dst, src2d):
        # HBM [512,512] f32 -> SBUF [128, 4*512] (j-chunk major), cast to bf16
        nc.gpsimd.dma_start(
            out=dst.rearrange("p (j w) -> p j w", j=4),
            in_=src2d.rearrange("(j p) w -> p j w", p=128))

    with tile.TileContext(nc) as tc:
        with (
            tc.tile_pool(name="consts", bufs=1) as consts,
            tc.tile_pool(name="shared", bufs=1) as shared,
            tc.tile_pool(name="chan", bufs=2) as chan,
            tc.tile_pool(name="psum", bufs=1, space="PSUM") as psum,
        ):
            G = consts.tile([128, 1280], bf16)
            nc.gpsimd.dma_start(out=G[:], in_=g_d)
            I = consts.tile([128, 2048], bf16)
            ld(I, img_d[0])

            def ap_blur(X):
                """pass A': psum tiles T1 (list of 4 [128,512] f32) = X^T B^T."""
                ps = [psum.tile([128, 512], f32, name=f"psA{wb}", tag=f"psA{wb}") for wb in range(4)]
                for j in range(4):
                    i0 = _BAND[j][0]
                    ilen = len(_BAND[j])
                    rhs = G[:, _GJ_OFF[j]:_GJ_OFF[j] + _GJ_LEN[j]]
                    for wb in range(4):
                        lhsT = X[:, 512 * j + 128 * wb: 512 * j + 128 * (wb + 1)]
                        nc.tensor.matmul(
                            ps[wb][:, 128 * i0: 128 * (i0 + ilen)],
                            lhsT, rhs, start=(j == 0), stop=(j == 3),
                            skip_group_check=True)
                return ps

            def handoff(ps, engines="AAAA"):
                """psum A' tiles -> one [128,2048] bf16 sbuf tile."""
                t = chan.tile([128, 2048], bf16, tag="t1")
                for wb in range(4):
                    dst = t[:, 512 * wb: 512 * (wb + 1)]
                    if engines[wb] == "A":
                        nc.scalar.copy(dst, ps[wb][:])
                    else:
                        nc.vector.tensor_copy(dst, ps[wb][:])
                return t

            def c_blur(T1, tag):
                """pass C: psum tiles out[i] [128,512] f32 = (B @ T1) chunks."""
                ps = [psum.tile([128, 512], f32, name=f"psC{i}", tag=f"psC{i}") for i in range(4)]
                for i in range(4):
                    band = _BAND[i]  # symmetric: band(i) as j-range equals _BAND[i]
                    for pos, j in enumerate(band):
                        off = _GJ_OFF[j] + 128 * (i - _BAND[j][0])
                        lhsT = G[:, off:off + 128]
                        rhs = T1[:, 512 * j: 512 * (j + 1)]
                        nc.tensor.matmul(
                            ps[i][:], lhsT, rhs,
                            start=(pos == 0), stop=(pos == len(band) - 1))
                return ps

            def blur2(X):
                return c_blur(handoff(ap_blur(X)), tag="c")

            # ---- shared (img) stage; all blur outputs are [W,H]-oriented ----
            I2 = shared.tile([128, 2048], bf16)
            nc.vector.tensor_mul(I2[:], I[:], I[:])
            psI = blur2(I)   # mI_raw^T
            mIs = shared.tile([128, 2048], bf16)   # true-scale mean_I^T
            for wb in range(4):
                nc.scalar.activation(mIs[:, 512 * wb:512 * (wb + 1)], psI[wb][:],
                                     Act.Copy, 0.0, U)
            psI2 = blur2(I2)  # corrI_raw^T
            m2 = shared.tile([128, 2048], f32)
            nc.vector.tensor_mul(m2[:], mIs[:], mIs[:])
            varps = shared.tile([128, 2048], f32)
            for wb in range(4):
                sl = slice(512 * wb, 512 * (wb + 1))
                # var = U*corrI_raw - mI^2   (merged psum evac)
                nc.vector.scalar_tensor_tensor(
                    varps[:, sl], psI2[wb][:], U, m2[:, sl],
                    op0=Alu.mult, op1=Alu.subtract)
            nc.vector.tensor_scalar_add(varps[:], varps[:], EPS)
            R = shared.tile([128, 2048], f32)
            nc.vector.reciprocal_approx_fast(R[:], varps[:])
            RS = shared.tile([128, 2048], bf16)
            nc.vector.tensor_scalar_mul(RS[:], R[:], U)     # U * R
            mIR = shared.tile([128, 2048], bf16)
            nc.vector.tensor_mul(mIR[:], mIs[:], R[:])      # mI * R

            # ---- per feature channel ----
            for d in range(D):
                Xd = chan.tile([128, 2048], bf16, tag="xd")
                ld(Xd, feat_d[d])
                Pd = chan.tile([128, 2048], bf16, tag="pd")
                nc.gpsimd.tensor_mul(Pd[:], Xd[:], I[:])

                ps_mp = blur2(Xd)       # mp_raw^T
                mp = chan.tile([128, 2048], bf16, tag="mp")
                for wb in range(4):     # true-scale mp (ACT, scale=U)
                    nc.scalar.activation(mp[:, 512 * wb:512 * (wb + 1)],
                                         ps_mp[wb][:], Act.Copy, 0.0, U)
                ps_cip = blur2(Pd)      # corrIp_raw^T
                t2 = chan.tile([128, 2048], bf16, tag="t2")
                for wb in range(4):     # t2 = corrIp * R  (merged evac)
                    sl = slice(512 * wb, 512 * (wb + 1))
                    nc.vector.tensor_mul(t2[:, sl], ps_cip[wb][:], RS[:, sl])
                t1 = chan.tile([128, 2048], bf16, tag="t1m")
                nc.vector.tensor_mul(t1[:], mp[:], mIR[:])
                a = chan.tile([128, 2048], bf16, tag="a")
                nc.vector.tensor_sub(a[:], t2[:], t1[:])
                u2 = chan.tile([128, 2048], bf16, tag="u2")
                nc.gpsimd.tensor_mul(u2[:], a[:], mIs[:])
                b = chan.tile([128, 2048], bf16, tag="b")
                nc.vector.tensor_sub(b[:], mp[:], u2[:])

                ps_ma = blur2(a)        # ma_raw, [H,W] again
                v = chan.tile([128, 2048], f32, tag="v")
                for wb in range(4):     # v = (U^2 * ma_raw) * I   (a true-scale => /121 once... )
                    sl = slice(512 * wb, 512 * (wb + 1))
                    nc.vector.scalar_tensor_tensor(
                        v[:, sl], ps_ma[wb][:], U, I[:, sl],
                        op0=Alu.mult, op1=Alu.mult)
                ps_mb = blur2(b)        # mb_raw
                o = chan.tile([128, 2048], bf16, tag="o")
                for wb in range(4):     # o = U*mb_raw + v
                    sl = slice(512 * wb, 512 * (wb + 1))
                    nc.vector.scalar_tensor_tensor(
                        o[:, sl], ps_mb[wb][:], U, v[:, sl],
                        op0=Alu.mult, op1=Alu.add)
                nc.gpsimd.dma_start(
                    out=out_d[d].rearrange("(j p) w -> p j w", p=128),
                    in_=o.rearrange("p (j w) -> p j w", j=4))

    nc.compile()
    return nc


_NC_CACHE = None


def kernel(feat: np.ndarray, img: np.ndarray) -> np.ndarray:
    global _NC_CACHE
    from concourse.bass_utils import run_bass_kernel_spmd

    if _NC_CACHE is None:
        _NC_CACHE = _build_bass()
    nc = _NC_CACHE
    g = _g_packed()
    feat = np.asarray(feat, np.float32)
    img = np.asarray(img, np.float32)
    in_maps = [
        {"feat": feat[c], "img": img[c], "gmat": g} for c in range(NCORES)
    ]
    res = run_bass_kernel_spmd(nc, in_maps, list(range(NCORES)))
    return np.stack([res.results[c]["out"] for c in range(NCORES)], axis=0)
